# revision 20
# baseline (speedup 1.0000x reference)
"""CapsNet forward kernel for Trainium2, 8-core data-parallel.

Strategy (per spec sharding_hint): batch (512) split across 8 cores (64 each);
all params replicated. Routing logits b are a batch-mean -> AllReduce of
per-core partial deltas (1152 floats) per routing round (rounds 1,2 only;
round 3's b update is dead in the reference).

v2: all big matmuls in fp16 (1 cycle/row on the PE vs 2 for fp32-HIGH),
batched conv1 im2col DMA, conv2 accumulates both ci-blocks in PSUM,
routing weights prefetched during conv, AllGather+local-sum replaced by
AllReduce, and a warm-up collective mid-conv absorbs inter-core skew so
the first real AllReduce doesn't eat ~40us of stall.

Math restructuring (keeps exact semantics, avoids materializing u):
  r := s*1152 + n  (s=caps idx, n=(c32,oy,ox))  == co*36 + pix  with co=s*32+c32
  xr2[b, r]   = primary-caps output (relu), flattened
  W2n[r, hl]  = W.transpose(3,0,1,2).reshape(9216,160)
  s[b,hl]  = sum_r c[n(r)] * W2n[r,hl] * xr2[b,r]        (matmul, K=9216)
  v        = squash_dim1(s)
  P[r,b]   = sum_hl W2n[r,hl] * v[b,hl]                  (matmul)
  delta[n] = 1/(B*160) * sum_s sum_b xr2[b,r]*P[r,b]     (DVE TT+reduce)
Convs are PE matmuls: conv1 via in-SBUF "wide patch" im2col (K=81),
primary-caps conv via 81 shifted-window matmuls accumulated in PSUM (K=256,
both 128-chunks accumulated in the same PSUM bank).
"""

import numpy as np

import concourse.bass as bass
import concourse.mybir as mybir
import concourse.tile as tile
from concourse.ap import AP
from concourse.bass_utils import run_bass_kernel_spmd

F32 = mybir.dt.float32
F16 = mybir.dt.float16
AL = mybir.AluOpType
AF = mybir.ActivationFunctionType
AX = mybir.AxisListType

NCORES = 8
B = 512
BC = B // NCORES           # 64 images per core
MAX_WAITS = 1              # walrus on this path allows 1 sync wait per inst
HL = 160                   # 10 classes x 16 pose
NS = 9216                  # 1152 caps x 8
NT = NS // 128             # 72 K-tiles
GROUPS = [(0, 14), (14, 14), (28, 14), (42, 14), (56, 8)]  # conv2 image groups
ROUTE_SCALE = 1.0 / (B * HL)
CHUNK = 8                  # conv1 images per im2col DMA chunk


def _r(t, dims):
    """Raw AP on tile/ap t with explicit [step, count] dims (elements)."""
    return AP(t.tensor, t.offset, dims)


def split_waits(nc, max_waits=MAX_WAITS):
    """This walrus build rejects >max_waits sync waits per instruction; move
    excess waits onto same-engine NoOps inserted immediately before."""
    for f in nc.m.functions:
        for blk in f.blocks:
            out = []
            for ins in blk.instructions:
                si = ins.sync_info
                if si is not None and si.on_wait and len(si.on_wait) > max_waits:
                    waits = list(si.on_wait)
                    k = 0
                    while len(waits) > max_waits:
                        chunk, waits = waits[:max_waits], waits[max_waits:]
                        nop = mybir.InstNoOp(name=f"{ins.name}-ws{k}", ins=[], outs=[])
                        nop.engine = ins.engine
                        nop.sync_info = mybir.SyncInfo(on_wait=chunk, on_update=[])
                        out.append(nop)
                        k += 1
                    ins.sync_info = mybir.SyncInfo(
                        on_wait=waits, on_update=list(si.on_update or []))
                out.append(ins)
            blk.instructions = out


def build_nc():
    nc = bass.Bass(num_devices=NCORES)

    xpatch = nc.dram_tensor("xpatch", [81, BC * 560], F16, kind="ExternalInput")
    w1t = nc.dram_tensor("w1t", [81, 256], F16, kind="ExternalInput")
    b1 = nc.dram_tensor("b1", [256], F32, kind="ExternalInput")
    pcwt = nc.dram_tensor("pcwt", [81, 256, 256], F16, kind="ExternalInput")
    pcb = nc.dram_tensor("pcb", [256], F32, kind="ExternalInput")
    w2n = nc.dram_tensor("w2n", [NS, HL], F16, kind="ExternalInput")
    w2nt = nc.dram_tensor("w2nt", [HL, NS], F16, kind="ExternalInput")
    eye64 = nc.dram_tensor("eye64", [BC, BC], F16, kind="ExternalInput")
    vout = nc.dram_tensor("vout", [BC, HL], F32, kind="ExternalOutput")

    pc_rd = nc.dram_tensor("pc_rd", [NS, BC], F16)    # [r, b]

    with tile.TileContext(nc) as tc:
        with (
            tc.tile_pool(name="pers", bufs=1) as pers,
            tc.tile_pool(name="dram", bufs=1, space="DRAM") as dpool,
        ):
            w1t_sb = pers.tile([81, 256], F16)
            nc.sync.dma_start(w1t_sb[:], w1t[:])
            b1_sb = pers.tile([128, 2], F32)
            nc.sync.dma_start(b1_sb[:], _r(b1[:], [[1, 128], [128, 2]]))
            pcb_sb = pers.tile([128, 2], F32)
            nc.sync.dma_start(pcb_sb[:], _r(pcb[:], [[1, 128], [128, 2]]))
            ones128 = pers.tile([128, 1], F32)
            nc.gpsimd.memset(ones128[:], 1.0)
            ones1 = pers.tile([1, 128], F32)
            nc.gpsimd.memset(ones1[:], 1.0)
            b9 = pers.tile([128, 9], F32)
            eye_sb = pers.tile([BC, BC], F16)
            nc.sync.dma_start(eye_sb[:], eye64[:])
            # routing s-weights: needed right at routing start -> prefetch now
            w2sb = pers.tile([128, NT * HL], F16)

            # ---------------- conv phase ----------------
            with (
                tc.tile_pool(name="h1p", bufs=1) as h1p,
                tc.tile_pool(name="w2cp", bufs=2) as w2cp,
            ):
                h1s = [h1p.tile([128, BC * 400], F16, tag=f"h1_{ci}",
                                name=f"h1_{ci}")
                       for ci in range(2)]

                def load_w2c(co, ci, eng):
                    t = w2cp.tile([128, 81 * 128], F16, tag="w2c")
                    eng.dma_start(
                        t[:],
                        AP(pcwt[:].tensor, ci * 128 * 256 + co * 128,
                           [[256, 128], [256 * 256, 81], [1, 128]]),
                    )
                    return t

                with (
                    tc.tile_pool(name="pwp", bufs=2) as pwp,
                    tc.tile_pool(name="ps1p", bufs=4, space="PSUM") as ps1p,
                ):
                    NCH = BC // CHUNK
                    pas = []
                    w2c0 = []

                    def load_chunk(k, eng):
                        pa = pwp.tile([81, CHUNK * 560], F16, tag="pa")
                        eng.dma_start(
                            pa[:],
                            AP(xpatch[:].tensor, k * CHUNK * 560,
                               [[BC * 560, 81], [1, CHUNK * 560]]),
                        )
                        return pa

                    pas.append(load_chunk(0, nc.sync))
                    pas.append(load_chunk(1, nc.scalar))
                    for k in range(NCH):
                        pa = pas[k]
                        pstep = pa.ap[0][0]
                        # images in pairs: 2 matmuls into one bank-padded
                        # PSUM tile, then ONE batched relu per (pair, ci) —
                        # alternating Scalar / DVE
                        for lp in range(CHUNK // 2):
                            gi = k * CHUNK + lp * 2
                            for ci in range(2):
                                ps = ps1p.tile([128, 1024], F32, tag="ps1")
                                pp = ps.ap[0][0]
                                for u in range(2):
                                    rhs = AP(pa.tensor,
                                             pa.offset + (lp * 2 + u) * 560,
                                             [[pstep, 81], [28, 20], [1, 20]])
                                    nc.tensor.matmul(
                                        AP(ps.tensor, ps.offset + u * 512,
                                           [[pp, 128], [1, 400]]),
                                        w1t_sb[:, ci * 128:(ci + 1) * 128],
                                        rhs,
                                        start=True, stop=True,
                                    )
                                dst = AP(h1s[ci].tensor,
                                         h1s[ci].offset + gi * 400,
                                         [[h1s[ci].ap[0][0], 128],
                                          [400, 2], [1, 400]])
                                src = _r(ps, [[pp, 128], [512, 2], [1, 400]])
                                if ci == 0:
                                    nc.scalar.activation(
                                        dst, src, AF.Relu,
                                        bias=b1_sb[:, ci:ci + 1],
                                    )
                                else:
                                    nc.vector.tensor_scalar(
                                        dst, src, b1_sb[:, ci:ci + 1], 0.0,
                                        AL.add, AL.max,
                                    )
                            if lp == 0 and k + 2 < NCH:
                                pas.append(load_chunk(
                                    k + 2, nc.sync if k % 2 == 0 else nc.scalar))
                            if lp == 1 and k == NCH - 3:
                                # prefetch conv2 weights + routing s-weights
                                # behind the last patch chunks
                                w2c0.append(load_w2c(0, 0, nc.sync))
                                w2c0.append(load_w2c(0, 1, nc.scalar))
                                nc.sync.dma_start(
                                    w2sb[:],
                                    AP(w2n[:].tensor, 0,
                                       [[HL, 128], [128 * HL, NT], [1, HL]]),
                                )

                # ---- conv2: 81 shifted matmuls, K=256 via 2 PSUM-accumulated
                # 128-chunks ----
                with (
                    tc.tile_pool(name="ps2p", bufs=1, space="PSUM") as ps2p,
                    tc.tile_pool(name="pc2p", bufs=2) as pc2p,
                ):
                    for co_blk in range(2):
                        w2cs = w2c0 if co_blk == 0 else \
                            [load_w2c(1, 0, nc.sync), load_w2c(1, 1, nc.scalar)]
                        pc2 = pc2p.tile([128, BC * 36], F16, tag="pc2")
                        p2 = pc2.ap[0][0]
                        for g, (g0, nb) in enumerate(GROUPS):
                            ps2 = ps2p.tile([128, nb * 36], F32, tag=f"ps2_{g}")
                            pstep = ps2.ap[0][0]
                            out4 = _r(ps2, [[pstep, 128], [36, nb], [6, 6], [1, 6]])
                            for ci in range(2):
                                h1 = h1s[ci]
                                hp = h1.ap[0][0]
                                for kk in range(81):
                                    ky, kx = divmod(kk, 9)
                                    rhs = AP(h1.tensor,
                                             h1.offset + g0 * 400 + ky * 20 + kx,
                                             [[hp, 128], [400, nb], [40, 6], [2, 6]])
                                    nc.tensor.matmul(
                                        out4,
                                        w2cs[ci][:, kk * 128:(kk + 1) * 128],
                                        rhs,
                                        start=(ci == 0 and kk == 0),
                                        stop=(ci == 1 and kk == 80),
                                    )
                            # bias+relu, write pix-major (col = pix*BC + b)
                            nc.scalar.activation(
                                AP(pc2.tensor, pc2.offset + g0,
                                   [[p2, 128], [1, nb], [BC, 36]]),
                                _r(ps2, [[pstep, 128], [36, nb], [1, 36]]),
                                AF.Relu,
                                bias=pcb_sb[:, co_blk:co_blk + 1],
                            )
                        # pc2 -> pc_rd[r, b] in DRAM (r = co*36 + pix)
                        (nc.sync if co_blk == 0 else nc.scalar).dma_start(
                            AP(pc_rd[:].tensor, co_blk * 128 * 36 * BC,
                               [[36 * BC, 128], [BC, 36], [1, BC]]),
                            _r(pc2, [[p2, 128], [BC, 36], [1, BC]]),
                        )
                        if co_blk == 0:
                            # warm-up collective: absorbs inter-core skew off
                            # the critical path so the first real AllReduce is
                            # fast. Depends on pc2 (fires near conv end).
                            cinw = dpool.tile([128, 1], F32, name="cinw")
                            coutw = dpool.tile([128, 1], F32, name="coutw",
                                               addr_space="Shared")
                            nc.gpsimd.dma_start(cinw[:], pc2[:, 0:1])
                            nc.gpsimd.collective_compute(
                                "AllReduce", AL.add,
                                replica_groups=[list(range(NCORES))],
                                ins=[cinw.opt()], outs=[coutw.opt()],
                            )

            # ---------------- routing phase ----------------
            with (
                tc.tile_pool(name="rsb", bufs=1) as rsb,
                tc.tile_pool(name="rnd", bufs=2) as rnd,
                tc.tile_pool(name="sps", bufs=1, space="PSUM") as sps,
                tc.tile_pool(name="gps", bufs=4, space="PSUM") as gps,
                tc.tile_pool(name="zps", bufs=1, space="PSUM") as zps,
            ):
                # W2n^T in two hl-chunks: (128, NT*128) + (32, NT*128)
                w2nt_a = rsb.tile([128, NT * 128], F16)
                nc.sync.dma_start(
                    w2nt_a[:],
                    AP(w2nt[:].tensor, 0, [[NS, 128], [128, NT], [1, 128]]),
                )
                w2nt_b = rsb.tile([32, NT * 128], F16)
                nc.scalar.dma_start(
                    w2nt_b[:],
                    AP(w2nt[:].tensor, 128 * NS, [[NS, 32], [128, NT], [1, 128]]),
                )
                # xrT in two halves so round-1 s-matmuls can start while the
                # second half (co_blk 1) is still landing
                NTH = NT // 2
                xrT_h = [rsb.tile([128, NTH * BC], F16, tag=f"xrT{h}",
                                  name=f"xrT{h}")
                         for h in range(2)]
                nc.sync.dma_start(
                    xrT_h[0][:],
                    AP(pc_rd[:].tensor, 0, [[BC, 128], [128 * BC, NTH], [1, BC]]),
                )
                nc.scalar.dma_start(
                    xrT_h[1][:],
                    AP(pc_rd[:].tensor, NTH * 128 * BC,
                       [[BC, 128], [128 * BC, NTH], [1, BC]]),
                )

                def xr_slice(t):
                    return xrT_h[t // NTH][:, (t % NTH) * BC:(t % NTH + 1) * BC]

                prod = rsb.tile([128, NT * BC], F16)

                def s_matmul():
                    s_ps = sps.tile([BC, HL], F32, tag="s_ps")
                    for t in range(NT):
                        nc.tensor.matmul(
                            s_ps[:],
                            xr_slice(t),
                            w2sb[:, t * HL:(t + 1) * HL],
                            start=(t == 0), stop=(t == NT - 1),
                        )
                    return s_ps

                def squash(s_sb, out_dtype):
                    sq = rnd.tile([BC, HL], F32, tag="sq")
                    nc.scalar.square(sq[:], s_sb[:])
                    n2 = rnd.tile([BC, 16], F32, tag="n2")
                    nc.vector.tensor_reduce(
                        n2[:].rearrange("a b -> a b ()"),
                        _r(sq, [[sq.ap[0][0], BC], [1, 16], [16, 10]]),
                        AX.X, AL.add,
                    )
                    rt = rnd.tile([BC, 16], F32, tag="rt")
                    nc.scalar.sqrt(rt[:], n2[:])
                    n2p1 = rnd.tile([BC, 16], F32, tag="n2p1")
                    nc.vector.tensor_scalar_add(n2p1[:], n2[:], 1.0)
                    rcp = rnd.tile([BC, 16], F32, tag="rcp")
                    nc.vector.reciprocal(rcp[:], n2p1[:])
                    f = rnd.tile([BC, 16], F32, tag="f")
                    nc.vector.tensor_tensor(f[:], rt[:], rcp[:], AL.mult)
                    v_sb = rnd.tile([BC, HL], out_dtype, tag="v_sb")
                    nc.vector.tensor_tensor(
                        _r(v_sb, [[v_sb.ap[0][0], BC], [16, 10], [1, 16]]),
                        _r(s_sb, [[s_sb.ap[0][0], BC], [16, 10], [1, 16]]),
                        _r(f, [[f.ap[0][0], BC], [0, 10], [1, 16]]),
                        AL.mult,
                    )
                    return v_sb

                def p_delta_update(v16, rnd_idx, rce9):
                    """delta via P[r,b] = sum_hl W2n[r,hl] v[b,hl] (PE), then
                    D[r] = sum_b xrT[r,b]*P[r,b] (DVE). If xrT is c-scaled,
                    divide delta9 by ce9 (rce9 ap) to undo."""
                    vt_ps = gps.tile([128, BC], F16, tag="vt_ps", bufs=1)
                    nc.tensor.transpose(vt_ps[:], v16[:, 0:128], eye_sb[:])
                    vt_a = rnd.tile([128, BC], F16, tag="vt_a")
                    nc.scalar.copy(vt_a[:], vt_ps[:])
                    vtb_ps = gps.tile([32, BC], F16, tag="vtb_ps", bufs=1)
                    nc.tensor.transpose(vtb_ps[:], v16[:, 128:160], eye_sb[:])
                    vt_b = rnd.tile([32, BC], F16, tag="vt_b")
                    nc.scalar.copy(vt_b[:], vtb_ps[:])
                    # P in 4-t batches; DVE multiplies straight out of PSUM
                    TB = 4
                    for tb in range(NT // TB):
                        p_ps = gps.tile([128, TB * BC], F32, tag="p_ps", bufs=3)
                        for j in range(TB):
                            t = tb * TB + j
                            nc.tensor.matmul(
                                p_ps[:, j * BC:(j + 1) * BC],
                                w2nt_a[:, t * 128:(t + 1) * 128],
                                vt_a[:],
                                start=True, stop=False,
                            )
                            nc.tensor.matmul(
                                p_ps[:, j * BC:(j + 1) * BC],
                                w2nt_b[:, t * 128:(t + 1) * 128],
                                vt_b[:],
                                start=False, stop=True,
                            )
                        t0 = tb * TB
                        xh = xrT_h[t0 // NTH]
                        nc.vector.tensor_tensor(
                            prod[:, t0 * BC:(t0 + TB) * BC],
                            AP(xh.tensor, xh.offset + (t0 % NTH) * BC,
                               [[xh.ap[0][0], 128], [1, TB * BC]]),
                            p_ps[:],
                            AL.mult,
                        )
                    D = rnd.tile([128, NT], F32, tag="D")
                    half = (NT // 2) * BC
                    for hx in range(2):
                        nc.vector.tensor_reduce(
                            D[:, hx * (NT // 2):(hx + 1) * (NT // 2)]
                            .rearrange("a b -> a b ()"),
                            AP(prod.tensor, prod.offset + hx * half,
                               [[prod.ap[0][0], 128], [BC, NT // 2], [1, BC]]),
                            AX.X, AL.add,
                        )
                    delta9 = rnd.tile([128, 9], F32, tag="delta9")
                    nc.vector.tensor_reduce(
                        delta9[:].rearrange("a b -> a b ()"),
                        _r(D, [[D.ap[0][0], 128], [1, 9], [9, 8]]),
                        AX.X, AL.add,
                    )
                    if rce9 is not None:
                        nc.vector.tensor_tensor(delta9[:], delta9[:], rce9[:], AL.mult)
                    cin = dpool.tile([128, 9], F32, name=f"cin{rnd_idx}")
                    cout = dpool.tile([128, 9], F32, name=f"cout{rnd_idx}",
                                      addr_space="Shared")
                    nc.gpsimd.dma_start(cin[:], delta9[:])
                    nc.gpsimd.collective_compute(
                        "AllReduce", AL.add,
                        replica_groups=[list(range(NCORES))],
                        ins=[cin.opt()], outs=[cout.opt()],
                    )
                    dsum = rnd.tile([128, 9], F32, tag="dsum")
                    nc.gpsimd.dma_start(dsum[:], cout[:])
                    if rnd_idx == 0:
                        nc.scalar.mul(b9[:], dsum[:], ROUTE_SCALE)
                    else:
                        sc = rnd.tile([128, 9], F32, tag="sc")
                        nc.scalar.mul(sc[:], dsum[:], ROUTE_SCALE)
                        nc.vector.tensor_tensor(b9[:], b9[:], sc[:], AL.add)

                def softmax_ce9():
                    """ce9[p,j] = softmax(b9)[n=j*128+p], F32 (128,9)."""
                    e9 = rnd.tile([128, 9], F32, tag="e9")
                    nc.scalar.activation(e9[:], b9[:], AF.Exp)
                    rs9 = rnd.tile([128, 1], F32, tag="rs9")
                    nc.vector.tensor_reduce(
                        rs9[:].rearrange("a b -> a b ()"), e9[:], AX.X, AL.add)
                    z_ps = zps.tile([1, 1], F32, tag="z_ps")
                    nc.tensor.matmul(z_ps[:], ones128[:], rs9[:], start=True, stop=True)
                    z_sb = rnd.tile([1, 1], F32, tag="z_sb")
                    nc.scalar.copy(z_sb[:], z_ps[:])
                    zb_ps = zps.tile([128, 1], F32, tag="zb_ps")
                    nc.tensor.matmul(zb_ps[:], ones1[:], z_sb[:], start=True, stop=True)
                    rz = rnd.tile([128, 1], F32, tag="rz")
                    nc.vector.reciprocal(rz[:], zb_ps[:])
                    ce9 = rnd.tile([128, 9], F32, tag="ce9")
                    nc.vector.tensor_scalar_mul(ce9[:], e9[:], rz[:])
                    return ce9

                def scale_xrT(m9f32):
                    """xrT[p, (q,j,b)] *= m9[p, j] in place (m9 cast to F16)."""
                    m16 = rnd.tile([128, 9], F16, tag="m16")
                    nc.scalar.copy(m16[:], m9f32[:])
                    for h in range(2):
                        xh = xrT_h[h]
                        nc.vector.tensor_tensor(
                            _r(xh, [[xh.ap[0][0], 128], [9 * BC, 4],
                                    [BC, 9], [1, BC]]),
                            _r(xh, [[xh.ap[0][0], 128], [9 * BC, 4],
                                    [BC, 9], [1, BC]]),
                            _r(m16, [[m16.ap[0][0], 128], [0, 4],
                                     [1, 9], [0, BC]]),
                            AL.mult,
                        )

                # ---- round 1 (c uniform; xrT unscaled) ----
                s_ps = s_matmul()
                s_sb = rnd.tile([BC, HL], F32, tag="s_sb")
                nc.scalar.mul(s_sb[:], s_ps[:], 1.0 / 1152.0)
                v16 = squash(s_sb, F16)
                p_delta_update(v16, 0, None)
                # ---- round 2 ----
                ce9_2 = softmax_ce9()
                scale_xrT(ce9_2)
                rce9 = rnd.tile([128, 9], F32, tag="rce9")
                nc.vector.reciprocal(rce9[:], ce9_2[:])
                s_ps = s_matmul()
                s_sb = rnd.tile([BC, HL], F32, tag="s_sb")
                nc.scalar.copy(s_sb[:], s_ps[:])
                v16 = squash(s_sb, F16)
                p_delta_update(v16, 1, rce9)
                # ---- round 3 (b update dead) ----
                ce9_3 = softmax_ce9()
                ratio9 = rnd.tile([128, 9], F32, tag="ratio9")
                nc.vector.tensor_tensor(ratio9[:], ce9_3[:], rce9[:], AL.mult)
                scale_xrT(ratio9)
                s_ps = s_matmul()
                s_sb = rnd.tile([BC, HL], F32, tag="s_sb")
                nc.scalar.copy(s_sb[:], s_ps[:])
                v_sb = squash(s_sb, F32)
                nc.sync.dma_start(vout[:], v_sb[:])

    return nc


_NC_CACHE = None


def _get_nc():
    global _NC_CACHE
    if _NC_CACHE is None:
        nc = build_nc()
        split_waits(nc)
        _NC_CACHE = nc
    return _NC_CACHE


def prepare_inputs(x, conv1_w, conv1_b, pc_w, pc_b, W):
    x = np.asarray(x, np.float32)
    xf = np.zeros((B, 800), np.float16)
    xf[:, :784] = x.reshape(B, 784).astype(np.float16)
    # host-side im2col ("wide patch"): xp[i, (ky,kx), j] = xf[i, 28*ky+kx+j]
    xp = np.lib.stride_tricks.as_strided(
        xf, shape=(B, 9, 9, 560), strides=(1600, 56, 2, 2)).reshape(B, 81, 560)
    w1t = np.ascontiguousarray(
        np.asarray(conv1_w, np.float32).reshape(256, 81).T).astype(np.float16)
    b1 = np.ascontiguousarray(np.asarray(conv1_b, np.float32))
    pcwt = np.ascontiguousarray(
        np.asarray(pc_w, np.float32).reshape(256, 256, 81).transpose(2, 1, 0)
    ).astype(np.float16)
    pcb = np.ascontiguousarray(np.asarray(pc_b, np.float32).reshape(256))
    w2n = np.ascontiguousarray(
        np.asarray(W, np.float32).transpose(3, 0, 1, 2).reshape(NS, HL)
    ).astype(np.float16)
    w2nt = np.ascontiguousarray(w2n.T)
    eye64 = np.eye(BC, dtype=np.float16)
    in_maps = []
    for c in range(NCORES):
        in_maps.append({
            "xpatch": np.ascontiguousarray(
                xp[c * BC:(c + 1) * BC].transpose(1, 0, 2).reshape(81, BC * 560)),
            "w1t": w1t, "b1": b1, "pcwt": pcwt, "pcb": pcb, "w2n": w2n,
            "w2nt": w2nt, "eye64": eye64,
        })
    return in_maps


def kernel(x, conv1_w, conv1_b, pc_w, pc_b, W, _trace=False, _trace_kwargs=None):
    nc = _get_nc()
    in_maps = prepare_inputs(x, conv1_w, conv1_b, pc_w, pc_b, W)
    res = run_bass_kernel_spmd(
        nc, in_maps, list(range(NCORES)),
        trace=_trace, **(_trace_kwargs or {}),
    )
    v = np.concatenate([np.asarray(res.results[c]["vout"]) for c in range(NCORES)], 0)
    out = v.reshape(B, 1, 1, 10, 16).astype(np.float32)
    if _trace:
        return out, res
    return out


# revision 33
# speedup vs baseline: 1.0485x; 1.0485x over previous
"""CapsNet forward kernel for Trainium2, 8-core data-parallel.

Strategy (per spec sharding_hint): batch (512) split across 8 cores (64 each);
all params replicated. Routing logits b are a batch-mean -> AllReduce of
per-core partial deltas (1152 floats) per routing round (rounds 1,2 only;
round 3's b update is dead in the reference).

v4: all big matmuls fp16 (1 cycle/row); host-side im2col for conv1 and
host-retiled weights so every weight DMA is contiguous (128 descriptors,
not ~10k); routing contraction tiled by (co_blk, pix) so it consumes the
primary-caps output directly from SBUF (no DRAM round-trip / scatter DMAs);
routing logits kept as [32, 36] (n = c32*36 + pix) with tiny mask-matmuls
for the cross-partition regroups; one AllReduce per round (rounds 1-2) and
a warm-up collective mid-conv to absorb inter-core skew.

Math (keeps exact semantics, never materializes u):
  n = c32*36 + pix, co = s*32 + c32, r = s*1152 + n
  xr[co, pix, b]   primary-caps relu output (SBUF, 2 co-halves)
  W2[co, pix, hl]  = W.transpose(3,0,1,2).reshape(9216,160) re-indexed
  s[b,hl]  = sum_{co,pix} c[n] * xr * W2          (72 PE matmuls, K=128)
  v        = squash_dim1(s)
  P[co,pix,b] = sum_hl W2 * v[b,hl]               (PE)
  delta[n] = 1/(B*160) * sum_{s,b} xr * P         (DVE + mask matmul)
Convs are PE matmuls: conv1 via host-im2col patches (K=81), primary-caps
conv via 81 shifted-window matmuls accumulated in PSUM (K=256 as 2x128).
"""

import numpy as np

import concourse.bass as bass
import concourse.mybir as mybir
import concourse.tile as tile
from concourse.ap import AP
from concourse.bass_utils import run_bass_kernel_spmd

F32 = mybir.dt.float32
F16 = mybir.dt.float16
AL = mybir.AluOpType
AF = mybir.ActivationFunctionType
AX = mybir.AxisListType

NCORES = 8
B = 512
BC = B // NCORES           # 64 images per core
MAX_WAITS = 1              # walrus on this path allows 1 sync wait per inst
HL = 160                   # 10 classes x 16 pose
GROUPS = [(0, 14), (14, 14), (28, 14), (42, 14), (56, 8)]  # conv2 image groups
ROUTE_SCALE = 1.0 / (B * HL)
CHUNK = 8                  # conv1 images per im2col DMA chunk


def _r(t, dims):
    """Raw AP on tile/ap t with explicit [step, count] dims (elements)."""
    return AP(t.tensor, t.offset, dims)


def split_waits(nc, max_waits=MAX_WAITS):
    """This walrus build rejects >max_waits sync waits per instruction; move
    excess waits onto same-engine NoOps inserted immediately before."""
    for f in nc.m.functions:
        for blk in f.blocks:
            out = []
            for ins in blk.instructions:
                si = ins.sync_info
                if si is not None and si.on_wait and len(si.on_wait) > max_waits:
                    waits = list(si.on_wait)
                    k = 0
                    while len(waits) > max_waits:
                        chunk, waits = waits[:max_waits], waits[max_waits:]
                        nop = mybir.InstNoOp(name=f"{ins.name}-ws{k}", ins=[], outs=[])
                        nop.engine = ins.engine
                        nop.sync_info = mybir.SyncInfo(on_wait=chunk, on_update=[])
                        out.append(nop)
                        k += 1
                    ins.sync_info = mybir.SyncInfo(
                        on_wait=waits, on_update=list(si.on_update or []))
                out.append(ins)
            blk.instructions = out


def build_nc():
    nc = bass.Bass(num_devices=NCORES)

    xpatch = nc.dram_tensor("xpatch", [81, BC * 560], F16, kind="ExternalInput")
    w1t = nc.dram_tensor("w1t", [81, 256], F16, kind="ExternalInput")
    b1 = nc.dram_tensor("b1", [256], F32, kind="ExternalInput")
    # conv2 weights, retiled: [co_blk, ci_blk, 128ci, 81kk * 128co]
    pcw4 = nc.dram_tensor("pcw4", [2, 2, 128, 81 * 128], F16,
                          kind="ExternalInput")
    pcb = nc.dram_tensor("pcb", [256], F32, kind="ExternalInput")
    # routing weights in (co_blk, pix) tiling
    w2p = nc.dram_tensor("w2p", [2, 128, 36 * HL], F16, kind="ExternalInput")
    w2pt_a = nc.dram_tensor("w2pt_a", [2, 128, 36 * 128], F16,
                            kind="ExternalInput")
    w2pt_b = nc.dram_tensor("w2pt_b", [2, 32, 36 * 128], F16,
                            kind="ExternalInput")
    eye64 = nc.dram_tensor("eye64", [BC, BC], F16, kind="ExternalInput")
    maskT = nc.dram_tensor("maskT", [128, 32], F32, kind="ExternalInput")
    mask16 = nc.dram_tensor("mask16", [32, 128], F16, kind="ExternalInput")
    vout = nc.dram_tensor("vout", [BC, HL], F32, kind="ExternalOutput")

    with tile.TileContext(nc) as tc:
        with (
            tc.tile_pool(name="pers", bufs=1) as pers,
            tc.tile_pool(name="dram", bufs=1, space="DRAM") as dpool,
        ):
            w1t_sb = pers.tile([81, 256], F16)
            nc.sync.dma_start(w1t_sb[:], w1t[:])
            b1_sb = pers.tile([128, 2], F32)
            nc.sync.dma_start(b1_sb[:], _r(b1[:], [[1, 128], [128, 2]]))
            pcb_sb = pers.tile([128, 2], F32)
            nc.sync.dma_start(pcb_sb[:], _r(pcb[:], [[1, 128], [128, 2]]))
            ones32 = pers.tile([32, 1], F32)
            nc.gpsimd.memset(ones32[:], 1.0)
            ones1 = pers.tile([1, 32], F32)
            nc.gpsimd.memset(ones1[:], 1.0)
            b32 = pers.tile([32, 36], F32)
            eye_sb = pers.tile([BC, BC], F16)
            nc.sync.dma_start(eye_sb[:], eye64[:])
            maskT_sb = pers.tile([128, 32], F32)
            nc.scalar.dma_start(maskT_sb[:], maskT[:])
            mask16_sb = pers.tile([32, 128], F16)
            nc.scalar.dma_start(mask16_sb[:], mask16[:])
            # routing s-weights [co_blk][128, 36*160]; prefetched during conv1
            w2p_sb = [pers.tile([128, 36 * HL], F16, tag=f"w2p{cb}",
                                name=f"w2p{cb}") for cb in range(2)]
            # xr (primary caps output), written by conv2 epilogue
            xr_sb = [pers.tile([128, 36 * BC], F16, tag=f"xr{cb}",
                               name=f"xr{cb}") for cb in range(2)]

            # ---------------- conv phase ----------------
            with (
                tc.tile_pool(name="h1p", bufs=1) as h1p,
                tc.tile_pool(name="w2cp", bufs=2) as w2cp,
            ):
                h1s = [h1p.tile([128, BC * 400], F16, tag=f"h1_{ci}",
                                name=f"h1_{ci}")
                       for ci in range(2)]

                def load_w2c(co, ci, eng):
                    t = w2cp.tile([128, 81 * 128], F16, tag="w2c")
                    eng.dma_start(
                        t[:],
                        AP(pcw4[:].tensor, (co * 2 + ci) * 128 * 81 * 128,
                           [[81 * 128, 128], [1, 81 * 128]]),
                    )
                    return t

                with (
                    tc.tile_pool(name="pwp", bufs=2) as pwp,
                    tc.tile_pool(name="ps1p", bufs=4, space="PSUM") as ps1p,
                ):
                    NCH = BC // CHUNK
                    pas = []
                    w2c0 = []

                    def load_chunk(k, eng):
                        pa = pwp.tile([81, CHUNK * 560], F16, tag="pa")
                        eng.dma_start(
                            pa[:],
                            AP(xpatch[:].tensor, k * CHUNK * 560,
                               [[BC * 560, 81], [1, CHUNK * 560]]),
                        )
                        return pa

                    pas.append(load_chunk(0, nc.sync))
                    pas.append(load_chunk(1, nc.scalar))
                    for k in range(NCH):
                        pa = pas[k]
                        pstep = pa.ap[0][0]
                        # image pairs: 2 matmuls into one bank-padded PSUM
                        # tile, then ONE batched relu per (pair, ci),
                        # alternating Scalar / DVE
                        for lp in range(CHUNK // 2):
                            gi = k * CHUNK + lp * 2
                            for ci in range(2):
                                ps = ps1p.tile([128, 1024], F32, tag="ps1")
                                pp = ps.ap[0][0]
                                for u in range(2):
                                    rhs = AP(pa.tensor,
                                             pa.offset + (lp * 2 + u) * 560,
                                             [[pstep, 81], [28, 20], [1, 20]])
                                    nc.tensor.matmul(
                                        AP(ps.tensor, ps.offset + u * 512,
                                           [[pp, 128], [1, 400]]),
                                        w1t_sb[:, ci * 128:(ci + 1) * 128],
                                        rhs,
                                        start=True, stop=True,
                                    )
                                dst = AP(h1s[ci].tensor,
                                         h1s[ci].offset + gi * 400,
                                         [[h1s[ci].ap[0][0], 128],
                                          [400, 2], [1, 400]])
                                src = _r(ps, [[pp, 128], [512, 2], [1, 400]])
                                if ci == 0:
                                    nc.scalar.activation(
                                        dst, src, AF.Relu,
                                        bias=b1_sb[:, ci:ci + 1],
                                    )
                                else:
                                    nc.vector.tensor_scalar(
                                        dst, src, b1_sb[:, ci:ci + 1], 0.0,
                                        AL.add, AL.max,
                                    )
                            if lp == 0 and k + 2 < NCH:
                                pas.append(load_chunk(
                                    k + 2, nc.sync if k % 2 == 0 else nc.scalar))
                            if lp == 1 and k == NCH - 3:
                                # prefetch conv2 + routing weights behind the
                                # last patch chunks (all contiguous DMAs now)
                                w2c0.append(load_w2c(0, 0, nc.sync))
                                w2c0.append(load_w2c(0, 1, nc.scalar))
                                for cb in range(2):
                                    nc.sync.dma_start(
                                        w2p_sb[cb][:],
                                        AP(w2p[:].tensor, cb * 128 * 36 * HL,
                                           [[36 * HL, 128], [1, 36 * HL]]),
                                    )

                # ---- conv2: 81 shifted matmuls, K=256 via 2 PSUM-accumulated
                # 128-chunks; epilogue writes xr (pix-major) straight to SBUF
                with tc.tile_pool(name="ps2p", bufs=1, space="PSUM") as ps2p:
                    for co_blk in range(2):
                        w2cs = w2c0 if co_blk == 0 else \
                            [load_w2c(1, 0, nc.sync), load_w2c(1, 1, nc.scalar)]
                        xr = xr_sb[co_blk]
                        p2 = xr.ap[0][0]
                        for g, (g0, nb) in enumerate(GROUPS):
                            ps2 = ps2p.tile([128, nb * 36], F32, tag=f"ps2_{g}")
                            pstep = ps2.ap[0][0]
                            out4 = _r(ps2, [[pstep, 128], [36, nb], [6, 6], [1, 6]])
                            for ci in range(2):
                                h1 = h1s[ci]
                                hp = h1.ap[0][0]
                                for kk in range(81):
                                    ky, kx = divmod(kk, 9)
                                    rhs = AP(h1.tensor,
                                             h1.offset + g0 * 400 + ky * 20 + kx,
                                             [[hp, 128], [400, nb], [40, 6], [2, 6]])
                                    nc.tensor.matmul(
                                        out4,
                                        w2cs[ci][:, kk * 128:(kk + 1) * 128],
                                        rhs,
                                        start=(ci == 0 and kk == 0),
                                        stop=(ci == 1 and kk == 80),
                                    )
                            # bias+relu, write pix-major (col = pix*BC + b)
                            nc.scalar.activation(
                                AP(xr.tensor, xr.offset + g0,
                                   [[p2, 128], [1, nb], [BC, 36]]),
                                _r(ps2, [[pstep, 128], [36, nb], [1, 36]]),
                                AF.Relu,
                                bias=pcb_sb[:, co_blk:co_blk + 1],
                            )
                        if co_blk == 0:
                            # warm-up collective: absorbs inter-core skew off
                            # the critical path so the first real AllReduce is
                            # fast. Depends on xr (fires near conv end).
                            cinw = dpool.tile([128, 1], F32, name="cinw")
                            coutw = dpool.tile([128, 1], F32, name="coutw",
                                               addr_space="Shared")
                            nc.gpsimd.dma_start(cinw[:], xr[:, 0:1])
                            nc.gpsimd.collective_compute(
                                "AllReduce", AL.add,
                                replica_groups=[list(range(NCORES))],
                                ins=[cinw.opt()], outs=[coutw.opt()],
                            )

            # ---------------- routing phase ----------------
            with (
                tc.tile_pool(name="rsb", bufs=1) as rsb,
                tc.tile_pool(name="rnd", bufs=2) as rnd,
                tc.tile_pool(name="sps", bufs=1, space="PSUM") as sps,
                tc.tile_pool(name="gps", bufs=4, space="PSUM") as gps,
                tc.tile_pool(name="zps", bufs=1, space="PSUM") as zps,
            ):
                # W2^T for the P matmuls: loaded at routing start (h1 freed)
                w2pt_a_sb = [rsb.tile([128, 36 * 128], F16, tag=f"w2pta{cb}",
                                      name=f"w2pta{cb}") for cb in range(2)]
                for cb in range(2):
                    (nc.sync if cb == 0 else nc.scalar).dma_start(
                        w2pt_a_sb[cb][:],
                        AP(w2pt_a[:].tensor, cb * 128 * 36 * 128,
                           [[36 * 128, 128], [1, 36 * 128]]),
                    )
                w2pt_b_sb = [rsb.tile([32, 36 * 128], F16, tag=f"w2ptb{cb}",
                                      name=f"w2ptb{cb}") for cb in range(2)]
                for cb in range(2):
                    (nc.sync if cb == 0 else nc.scalar).dma_start(
                        w2pt_b_sb[cb][:],
                        AP(w2pt_b[:].tensor, cb * 32 * 36 * 128,
                           [[36 * 128, 32], [1, 36 * 128]]),
                    )
                prod = rsb.tile([128, 2 * 36 * BC], F16)

                def s_matmul():
                    s_ps = sps.tile([BC, HL], F32, tag="s_ps")
                    first, last = (0, 0), (1, 35)
                    for cb in range(2):
                        for pix in range(36):
                            nc.tensor.matmul(
                                s_ps[:],
                                xr_sb[cb][:, pix * BC:(pix + 1) * BC],
                                w2p_sb[cb][:, pix * HL:(pix + 1) * HL],
                                start=((cb, pix) == first),
                                stop=((cb, pix) == last),
                            )
                    return s_ps

                def squash(s_sb, out_dtype):
                    sq = rnd.tile([BC, HL], F32, tag="sq")
                    nc.scalar.square(sq[:], s_sb[:])
                    n2 = rnd.tile([BC, 16], F32, tag="n2")
                    nc.vector.tensor_reduce(
                        n2[:].rearrange("a b -> a b ()"),
                        _r(sq, [[sq.ap[0][0], BC], [1, 16], [16, 10]]),
                        AX.X, AL.add,
                    )
                    rt = rnd.tile([BC, 16], F32, tag="rt")
                    nc.scalar.sqrt(rt[:], n2[:])
                    n2p1 = rnd.tile([BC, 16], F32, tag="n2p1")
                    nc.vector.tensor_scalar_add(n2p1[:], n2[:], 1.0)
                    rcp = rnd.tile([BC, 16], F32, tag="rcp")
                    nc.vector.reciprocal(rcp[:], n2p1[:])
                    f = rnd.tile([BC, 16], F32, tag="f")
                    nc.vector.tensor_tensor(f[:], rt[:], rcp[:], AL.mult)
                    v_sb = rnd.tile([BC, HL], out_dtype, tag="v_sb")
                    nc.vector.tensor_tensor(
                        _r(v_sb, [[v_sb.ap[0][0], BC], [16, 10], [1, 16]]),
                        _r(s_sb, [[s_sb.ap[0][0], BC], [16, 10], [1, 16]]),
                        _r(f, [[f.ap[0][0], BC], [0, 10], [1, 16]]),
                        AL.mult,
                    )
                    return v_sb

                def p_delta_update(v16, rnd_idx, rce32):
                    """delta via P[co,pix,b] = sum_hl W2*v (PE), then
                    D[co,pix] = sum_b xr*P (DVE), then delta32[c32,pix] =
                    mask-matmul partition regroup. If xr is c-scaled, divide
                    by ce32 (rce32 ap) to undo."""
                    vt_ps = gps.tile([128, 2 * BC], F16, tag="vt_ps", bufs=1)
                    nc.tensor.transpose(vt_ps[:, 0:BC], v16[:, 0:128], eye_sb[:])
                    nc.tensor.transpose(
                        AP(vt_ps.tensor, vt_ps.offset + BC,
                           [[vt_ps.ap[0][0], 32], [1, BC]]),
                        v16[:, 128:160], eye_sb[:])
                    vt_a = rnd.tile([128, BC], F16, tag="vt_a")
                    nc.scalar.copy(vt_a[:], vt_ps[:, 0:BC])
                    vt_b = rnd.tile([32, BC], F16, tag="vt_b")
                    nc.scalar.copy(
                        vt_b[:],
                        AP(vt_ps.tensor, vt_ps.offset + BC,
                           [[vt_ps.ap[0][0], 32], [1, BC]]))
                    # P in 4-pix batches; DVE multiplies straight out of PSUM
                    TB = 4
                    for cb in range(2):
                        for pb in range(36 // TB):
                            p_ps = gps.tile([128, TB * BC], F32, tag="p_ps",
                                            bufs=2)
                            for j in range(TB):
                                pix = pb * TB + j
                                nc.tensor.matmul(
                                    p_ps[:, j * BC:(j + 1) * BC],
                                    w2pt_a_sb[cb][:, pix * 128:(pix + 1) * 128],
                                    vt_a[:],
                                    start=True, stop=False,
                                )
                                nc.tensor.matmul(
                                    p_ps[:, j * BC:(j + 1) * BC],
                                    w2pt_b_sb[cb][:, pix * 128:(pix + 1) * 128],
                                    vt_b[:],
                                    start=False, stop=True,
                                )
                            xh = xr_sb[cb]
                            nc.vector.tensor_tensor(
                                prod[:, (cb * 36 + pb * TB) * BC:
                                     (cb * 36 + pb * TB + TB) * BC],
                                AP(xh.tensor, xh.offset + pb * TB * BC,
                                   [[xh.ap[0][0], 128], [1, TB * BC]]),
                                p_ps[:],
                                AL.mult,
                            )
                    ds_ps = gps.tile([32, 36], F32, tag="ds_ps", bufs=1)
                    for cb in range(2):
                        D = rnd.tile([128, 36], F32, tag=f"D{cb}")
                        nc.vector.tensor_reduce(
                            D[:].rearrange("a b -> a b ()"),
                            AP(prod.tensor, prod.offset + cb * 36 * BC,
                               [[prod.ap[0][0], 128], [BC, 36], [1, BC]]),
                            AX.X, AL.add,
                        )
                        # regroup: delta32[c32,pix] = sum_{p: p%32==c32} D[p,pix]
                        nc.tensor.matmul(
                            ds_ps[:], maskT_sb[:], D[:],
                            start=(cb == 0), stop=(cb == 1),
                        )
                    delta32 = rnd.tile([32, 36], F32, tag="delta32")
                    if rce32 is not None:
                        nc.vector.tensor_tensor(
                            delta32[:], ds_ps[:], rce32[:], AL.mult)
                    else:
                        nc.scalar.copy(delta32[:], ds_ps[:])
                    cin = dpool.tile([32, 36], F32, name=f"cin{rnd_idx}")
                    cout = dpool.tile([32, 36], F32, name=f"cout{rnd_idx}",
                                      addr_space="Shared")
                    nc.gpsimd.dma_start(cin[:], delta32[:])
                    nc.gpsimd.collective_compute(
                        "AllReduce", AL.add,
                        replica_groups=[list(range(NCORES))],
                        ins=[cin.opt()], outs=[cout.opt()],
                    )
                    dsum = rnd.tile([32, 36], F32, tag="dsum")
                    nc.gpsimd.dma_start(dsum[:], cout[:])
                    if rnd_idx == 0:
                        nc.scalar.mul(b32[:], dsum[:], ROUTE_SCALE)
                    else:
                        sc = rnd.tile([32, 36], F32, tag="sc")
                        nc.scalar.mul(sc[:], dsum[:], ROUTE_SCALE)
                        nc.vector.tensor_tensor(b32[:], b32[:], sc[:], AL.add)

                def softmax_ce():
                    """ce32[c32,pix] = softmax(b32)[n=c32*36+pix], F32."""
                    e32 = rnd.tile([32, 36], F32, tag="e32")
                    nc.scalar.activation(e32[:], b32[:], AF.Exp)
                    rs = rnd.tile([32, 1], F32, tag="rs")
                    nc.vector.tensor_reduce(
                        rs[:].rearrange("a b -> a b ()"), e32[:], AX.X, AL.add)
                    z_ps = zps.tile([1, 1], F32, tag="z_ps")
                    nc.tensor.matmul(z_ps[:], ones32[:], rs[:], start=True, stop=True)
                    z_sb = rnd.tile([1, 1], F32, tag="z_sb")
                    nc.scalar.copy(z_sb[:], z_ps[:])
                    zb_ps = zps.tile([32, 1], F32, tag="zb_ps")
                    nc.tensor.matmul(zb_ps[:], ones1[:], z_sb[:], start=True, stop=True)
                    rz = rnd.tile([32, 1], F32, tag="rz")
                    nc.vector.reciprocal(rz[:], zb_ps[:])
                    ce32 = rnd.tile([32, 36], F32, tag="ce32")
                    nc.vector.tensor_scalar_mul(ce32[:], e32[:], rz[:])
                    return ce32

                def scale_xr(m32f32):
                    """xr[co, pix, b] *= m32[co%32, pix] in place."""
                    m16 = rnd.tile([32, 36], F16, tag="m16")
                    nc.scalar.copy(m16[:], m32f32[:])
                    cm_ps = sps.tile([128, 36], F32, tag="cm_ps")
                    nc.tensor.matmul(cm_ps[:], mask16_sb[:], m16[:],
                                     start=True, stop=True)
                    cmap = rnd.tile([128, 36], F16, tag="cmap")
                    nc.scalar.copy(cmap[:], cm_ps[:])
                    for cb in range(2):
                        xh = xr_sb[cb]
                        nc.vector.tensor_tensor(
                            _r(xh, [[xh.ap[0][0], 128], [BC, 36], [1, BC]]),
                            _r(xh, [[xh.ap[0][0], 128], [BC, 36], [1, BC]]),
                            _r(cmap, [[cmap.ap[0][0], 128], [1, 36], [0, BC]]),
                            AL.mult,
                        )

                # ---- round 1 (c uniform; xr unscaled) ----
                s_ps = s_matmul()
                s_sb = rnd.tile([BC, HL], F32, tag="s_sb")
                nc.scalar.mul(s_sb[:], s_ps[:], 1.0 / 1152.0)
                v16 = squash(s_sb, F16)
                p_delta_update(v16, 0, None)
                # ---- round 2 ----
                ce2 = softmax_ce()
                scale_xr(ce2)
                rce32 = rnd.tile([32, 36], F32, tag="rce32")
                nc.vector.reciprocal(rce32[:], ce2[:])
                s_ps = s_matmul()
                s_sb = rnd.tile([BC, HL], F32, tag="s_sb")
                nc.scalar.copy(s_sb[:], s_ps[:])
                v16 = squash(s_sb, F16)
                p_delta_update(v16, 1, rce32)
                # ---- round 3 (b update dead) ----
                ce3 = softmax_ce()
                ratio32 = rnd.tile([32, 36], F32, tag="ratio32")
                nc.vector.tensor_tensor(ratio32[:], ce3[:], rce32[:], AL.mult)
                scale_xr(ratio32)
                s_ps = s_matmul()
                s_sb = rnd.tile([BC, HL], F32, tag="s_sb")
                nc.scalar.copy(s_sb[:], s_ps[:])
                v_sb = squash(s_sb, F32)
                nc.sync.dma_start(vout[:], v_sb[:])

    return nc


_NC_CACHE = None


def _get_nc():
    global _NC_CACHE
    if _NC_CACHE is None:
        nc = build_nc()
        split_waits(nc)
        _NC_CACHE = nc
    return _NC_CACHE


def prepare_inputs(x, conv1_w, conv1_b, pc_w, pc_b, W):
    x = np.asarray(x, np.float32)
    xf = np.zeros((B, 800), np.float16)
    xf[:, :784] = x.reshape(B, 784).astype(np.float16)
    # host-side im2col ("wide patch"): xp[i, (ky,kx), j] = xf[i, 28*ky+kx+j]
    xp = np.lib.stride_tricks.as_strided(
        xf, shape=(B, 9, 9, 560), strides=(1600, 56, 2, 2)).reshape(B, 81, 560)
    w1t = np.ascontiguousarray(
        np.asarray(conv1_w, np.float32).reshape(256, 81).T).astype(np.float16)
    b1 = np.ascontiguousarray(np.asarray(conv1_b, np.float32))
    # pcw4[co_blk, ci_blk, ci128, kk*128co] = pc_w[co, ci, ky, kx]
    pcw = np.asarray(pc_w, np.float32).reshape(256, 256, 81)  # [co, ci, kk]
    pcw4 = np.ascontiguousarray(
        pcw.reshape(2, 128, 2, 128, 81).transpose(0, 2, 3, 4, 1)
    ).astype(np.float16)  # [co_blk, ci_blk, ci128, kk, co128]
    pcb = np.ascontiguousarray(np.asarray(pc_b, np.float32).reshape(256))
    # W2cp[co, pix, hl] = W2n[co*36+pix, hl]
    w2n = np.asarray(W, np.float32).transpose(3, 0, 1, 2).reshape(9216, HL)
    w2cp = w2n.reshape(256, 36, HL)
    w2p = np.ascontiguousarray(
        w2cp.reshape(2, 128, 36 * HL)).astype(np.float16)
    w2t = w2cp.transpose(2, 1, 0)                 # [hl, pix, co]
    w2pt_a = np.ascontiguousarray(
        w2t[:128].reshape(128, 36, 2, 128).transpose(2, 0, 1, 3)
        .reshape(2, 128, 36 * 128)).astype(np.float16)
    w2pt_b = np.ascontiguousarray(
        w2t[128:].reshape(32, 36, 2, 128).transpose(2, 0, 1, 3)
        .reshape(2, 32, 36 * 128)).astype(np.float16)
    eye64 = np.eye(BC, dtype=np.float16)
    maskT = np.zeros((128, 32), np.float32)
    maskT[np.arange(128), np.arange(128) % 32] = 1.0
    mask16 = np.ascontiguousarray(maskT.T).astype(np.float16)
    in_maps = []
    for c in range(NCORES):
        in_maps.append({
            "xpatch": np.ascontiguousarray(
                xp[c * BC:(c + 1) * BC].transpose(1, 0, 2).reshape(81, BC * 560)),
            "w1t": w1t, "b1": b1, "pcw4": pcw4, "pcb": pcb, "w2p": w2p,
            "w2pt_a": w2pt_a, "w2pt_b": w2pt_b, "eye64": eye64,
            "maskT": maskT, "mask16": mask16,
        })
    return in_maps


def kernel(x, conv1_w, conv1_b, pc_w, pc_b, W, _trace=False, _trace_kwargs=None):
    nc = _get_nc()
    in_maps = prepare_inputs(x, conv1_w, conv1_b, pc_w, pc_b, W)
    res = run_bass_kernel_spmd(
        nc, in_maps, list(range(NCORES)),
        trace=_trace, **(_trace_kwargs or {}),
    )
    v = np.concatenate([np.asarray(res.results[c]["vout"]) for c in range(NCORES)], 0)
    out = v.reshape(B, 1, 1, 10, 16).astype(np.float32)
    if _trace:
        return out, res
    return out
